# revision 1
# baseline (speedup 1.0000x reference)
"""FM layer (first + second order) on 8 TRN2 NeuronCores — dma_gather version.

Batch-parallel (512 rows/core). Table rows [w | V^T] are fp32 padded to 32
values (128B); a 512B block holds 4 rows. One dma_gather per field fetches
each batch row's block via int16 block-in-field indices; the 26 gathers ride
SWDGE queues 1-3 (queue 0 holds the issuing engine synchronously, 1-3 are
async ucode streams). Row-within-block (class c = idx%4) is resolved by 4
masked passes: tmp_c = G_c * mask_c (DVE), Se = sum_c sum_f tmp_c,
Ssq = sum_c sum_{f,k} tmp_c^2 (Act square+accum). Dense part and the final
combine reuse the baseline's packed-matmul trick.
"""

import os
import sys

sys.path.insert(0, "/opt/trn_rl_repo")

import numpy as np

import concourse.bass as bass
import concourse.bacc as bacc
import concourse.mybir as mybir
import concourse.tile as tile
from concourse import library_config
from concourse.ap import AP

N_DENSE = 13
N_FIELDS = 26
PER_FIELD = 100000
NROWS = N_FIELDS * PER_FIELD  # device table: sparse rows only
K = 16
BATCH = 4096
N_CORES = 8
BL = BATCH // N_CORES  # 512
P = 128
T = BL // P  # 4
ROW = 1 + K  # 17
PAD = 32  # f32 values per row (128B)
RPB = 4  # rows per 512B block
NBLK = PER_FIELD // RPB  # 25000
EW = RPB * PAD  # 128 f32 words per block
C = RPB
FHS = [16, 10]  # halves (for reduce/square granularity)
F0S = [0, 16]
MOFF = [0, 4 * 4 * 16]  # mask block offsets: h1 blocks start after h0's C*T*16
# gather quarters: TTs start as soon as each quarter lands
FQS = [8, 8, 5, 5]
FQ0 = [0, 8, 16, 21]
QH = [0, 0, 1, 1]  # which half each quarter belongs to

KM = 2 * N_DENSE + 1
NO = ROW + 1

F32 = mybir.dt.float32
I16 = mybir.dt.int16

def split_multiwaits(nc: bass.Bass, max_waits: int = 1) -> int:
    """This container's walrus encodes at most one sync-wait per instruction
    (setupSyncWait raises 'Too many sync wait commands' otherwise). Hoist
    extra waits into standalone EventSemaphore ops on the same engine.
    Each hoisted op incs a dedicated dummy sem nothing waits on (CoreSim
    requires EventSemaphore instructions to carry an update)."""
    import bass_rust

    # Tile assigns its sems (ids ~151-168) outside bass's free pool, so pick
    # the first bass-free id above everything Tile used.
    used = set()
    for func in nc.m.functions:
        for bb in func.blocks:
            for ins in bb.instructions:
                si = getattr(ins, "sync_info", None)
                if si:
                    for x in list(si.on_wait or []) + list(si.on_update or []):
                        used.add(x.id)
    dummy = None
    for num in range(max(used, default=0) + 1, 256):
        try:
            dummy = nc.alloc_semaphore("splitw_dummy", num=num)
            break
        except AssertionError:
            continue
    assert dummy is not None, "no free semaphore for splitw_dummy"
    n = 0
    for func in nc.m.functions:
        for bb in func.blocks:
            out = []
            for ins in bb.instructions:
                si = getattr(ins, "sync_info", None)
                if (
                    si is not None
                    and si.on_wait is not None
                    and len(si.on_wait) > max_waits
                ):
                    for w in list(si.on_wait[:-max_waits]):
                        n += 1
                        ev = mybir.InstEventSemaphore(
                            name=f"splitw_{n}", engine=ins.engine
                        )
                        ev.sync_info = mybir.SyncInfo(on_wait=[w], on_update=[])
                        bass_rust.then_inc(ev, dummy, 1, True)
                        out.append(ev)
                    ins.sync_info = mybir.SyncInfo(
                        on_wait=list(si.on_wait[-max_waits:]),
                        on_update=list(si.on_update or []),
                    )
                out.append(ins)
            bb.instructions = out
    return n




def build_nc() -> bass.Bass:
    nc = bacc.Bacc("TRN2", num_swdge_queues=4)

    table = nc.dram_tensor("table", [NROWS, PAD], F32, kind="ExternalInput")
    idx = nc.dram_tensor("idx", [128, N_FIELDS * (BL // 16)], I16, kind="ExternalInput")
    dmat = nc.dram_tensor("dmat", [KM, BL + NO], F32, kind="ExternalInput")
    msk = nc.dram_tensor("msk", [128, C * T * N_FIELDS], F32, kind="ExternalInput")
    out = nc.dram_tensor("out", [P, T], F32, kind="ExternalOutput")

    # load the gather ucode first so the swap overlaps kernel startup
    nc.gpsimd.load_library(library_config.mlp)

    with tile.TileContext(nc) as tc:
        with (
            tc.tile_pool(name="const", bufs=1) as cp,
            tc.tile_pool(name="sbuf", bufs=2) as sp,
            tc.tile_pool(name="psum", bufs=2, space="PSUM") as pp,
        ):
            idx_t = cp.tile([128, N_FIELDS * (BL // 16)], I16)
            nc.sync.dma_start(idx_t[:], idx[:])
            dmat_t = cp.tile([KM, BL + NO], F32)
            nc.sync.dma_start(dmat_t[:], dmat[:])
            msk_t = cp.tile([128, C * T * N_FIELDS], F32)
            nc.sync.dma_start(msk_t[:], msk[:])
            out_t = cp.tile([P, T], F32)

            mm_all = pp.tile([P, T * NO], F32)
            for t in range(T):
                nc.tensor.matmul(
                    mm_all[:, t * NO : (t + 1) * NO],
                    dmat_t[:, t * P : (t + 1) * P],
                    dmat_t[:, BL : BL + NO],
                    start=True,
                    stop=True,
                )

            # gathers: halves as separate tiles for gather/compute overlap
            HQ = [
                sp.tile([128, FQS[q] * 512], F32, tag=f"HQ{q}", name=f"HQ{q}", bufs=1)
                for q in range(4)
            ]
            for q in range(4):
                for fr in range(FQS[q]):
                    f = FQ0[q] + fr
                    nc.gpsimd.dma_gather(
                        out_ap=HQ[q][:, fr * 512 : (fr + 1) * 512].rearrange(
                            "p (s e) -> p s e", e=EW
                        ),
                        in_ap=AP(table, f * PER_FIELD * PAD, [[EW, NBLK], [1, EW]]),
                        idxs_ap=idx_t[:, f * (BL // 16) : (f + 1) * (BL // 16)],
                        num_idxs=BL,
                        num_idxs_reg=BL,
                        elem_size=EW,
                        elem_step=EW,
                        queue_num=1 + (f % 3),
                    )

            # masked extraction + reductions per (half, class)
            # H word(fr, t, c, j) = fr*512 + t*128 + c*32 + j
            S_parts = sp.tile([P, 2 * C * T * ROW], F32, tag="sparts", bufs=1)
            SQ_parts = sp.tile([P, 2 * C * T], F32, tag="sqparts", bufs=1)
            tmps = {}
            for h in range(2):
                FH = FHS[h]
                for c in range(C):
                    tmps[(h, c)] = sp.tile(
                        [P, T * FH * ROW], F32, tag=f"tmp{h}{c}", name=f"tmp{h}{c}"
                    )
            for q in range(4):
                h = QH[q]
                FH = FHS[h]
                FQ = FQS[q]
                qoff = FQ0[q] - F0S[h]  # field offset within the half
                for c in range(C):
                    tmp = tmps[(h, c)]
                    in0 = AP(
                        HQ[q].tensor,
                        HQ[q][:].offset + c * PAD,
                        [HQ[q][:].ap[0], [128, T], [512, FQ], [1, ROW]],
                    )
                    in1 = AP(
                        msk_t.tensor,
                        msk_t[:].offset + MOFF[h] + c * T * FH + qoff,
                        [msk_t[:].ap[0], [FH, T], [1, FQ], [0, ROW]],
                    )
                    # tmp layout [t, fr, j]; this TT fills fr in [qoff, qoff+FQ)
                    nc.vector.tensor_tensor(
                        out=AP(
                            tmp.tensor,
                            tmp[:].offset + qoff * ROW,
                            [tmp[:].ap[0], [FH * ROW, T], [ROW, FQ], [1, ROW]],
                        ),
                        in0=in0,
                        in1=in1,
                        op=mybir.AluOpType.mult,
                    )
            for h in range(2):
                FH = FHS[h]
                for c in range(C):
                    tmp = tmps[(h, c)]
                    nc.vector.tensor_reduce(
                        out=S_parts[
                            :, (h * C + c) * T * ROW : (h * C + c + 1) * T * ROW
                        ].rearrange("p (t j) -> p t j", t=T),
                        in_=AP(
                            tmp.tensor,
                            tmp[:].offset,
                            [tmp[:].ap[0], [FH * ROW, T], [1, ROW], [ROW, FH]],
                        ),
                        axis=mybir.AxisListType.X,
                        op=mybir.AluOpType.add,
                    )
                    for t in range(T):
                        sqd = sp.tile(
                            [P, FH * K], F32, tag="sqd", name=f"sqd{h}{c}{t}"
                        )
                        nc.scalar.activation(
                            out=sqd[:],
                            in_=AP(
                                tmp.tensor,
                                tmp[:].offset + t * FH * ROW + 1,
                                [tmp[:].ap[0], [ROW, FH], [1, K]],
                            ),
                            func=mybir.ActivationFunctionType.Square,
                            accum_out=SQ_parts[
                                :, (h * C + c) * T + t : (h * C + c) * T + t + 1
                            ],
                        )

            # combine partials: Se = sum over 8 (h,c); Ssq likewise
            Se = sp.tile([P, T * ROW], F32, tag="Se", bufs=1)
            nc.vector.tensor_tensor(
                out=Se[:],
                in0=S_parts[:, 0 : T * ROW],
                in1=S_parts[:, T * ROW : 2 * T * ROW],
                op=mybir.AluOpType.add,
            )
            for i in range(2, 2 * C):
                nc.vector.tensor_tensor(
                    out=Se[:],
                    in0=Se[:],
                    in1=S_parts[:, i * T * ROW : (i + 1) * T * ROW],
                    op=mybir.AluOpType.add,
                )
            Ssq = sp.tile([P, T], F32, tag="Ssq", bufs=1)
            nc.vector.tensor_reduce(
                out=Ssq[:].rearrange("p (t o) -> p t o", o=1),
                in_=AP(
                    SQ_parts.tensor,
                    SQ_parts[:].offset,
                    [SQ_parts[:].ap[0], [1, T], [T, 2 * C]],
                ),
                axis=mybir.AxisListType.X,
                op=mybir.AluOpType.add,
            )

            # final FM combine per tile (baseline math)
            for t in range(T):
                mm = mm_all[:, t * NO : (t + 1) * NO]
                ts = sp.tile([P, ROW], F32, tag="ts", name=f"ts{t}")
                nc.vector.tensor_tensor(
                    out=ts[:],
                    in0=Se[:, t * ROW : (t + 1) * ROW],
                    in1=mm[:, 0:ROW],
                    op=mybir.AluOpType.add,
                )
                se2 = sp.tile([P, 1], F32, tag="se2", name=f"se2{t}")
                sq2 = sp.tile([P, K], F32, tag="sq2", name=f"sq2{t}")
                nc.scalar.activation(
                    out=sq2[:],
                    in_=ts[:, 1:ROW],
                    func=mybir.ActivationFunctionType.Square,
                    accum_out=se2[:],
                )
                d1 = sp.tile([P, 1], F32, tag="d1", name=f"d1{t}")
                nc.vector.tensor_tensor(
                    out=d1[:],
                    in0=se2[:],
                    in1=Ssq[:, t : t + 1],
                    op=mybir.AluOpType.subtract,
                )
                d2 = sp.tile([P, 1], F32, tag="d2", name=f"d2{t}")
                nc.vector.tensor_tensor(
                    out=d2[:],
                    in0=d1[:],
                    in1=mm[:, ROW : ROW + 1],
                    op=mybir.AluOpType.subtract,
                )
                nc.vector.tensor_scalar(
                    out=out_t[:, t : t + 1],
                    in0=d2[:],
                    scalar1=0.5,
                    scalar2=ts[:, 0:1],
                    op0=mybir.AluOpType.mult,
                    op1=mybir.AluOpType.add,
                )
            nc.sync.dma_start(out[:], out_t[:])

    split_multiwaits(nc)
    nc.compile()
    return nc


def prepare_table(w, V):
    """fp32 device table [NROWS, 32]: row j = [w[13+j], V[:,13+j], pad].

    Rebuilt on every call: caching on first-call contents would silently
    return stale rows if the kernel is invoked again with different w/V."""
    tb = np.zeros((NROWS, PAD), dtype=np.float32)
    tb[:, 0] = w[N_DENSE:, 0]
    tb[:, 1 : 1 + K] = V[:, N_DENSE:].T
    return tb


def prepare_inputs(dense_inputs, sparse_inputs, w0, w, V):
    dense = np.asarray(dense_inputs, dtype=np.float32)
    sparse = np.asarray(sparse_inputs, dtype=np.int64)  # [B, 26] in [0, 1e5)
    w0 = np.asarray(w0, dtype=np.float32).reshape(-1)
    w = np.asarray(w, dtype=np.float32).reshape(-1, 1)
    V = np.asarray(V, dtype=np.float32)

    table = prepare_table(w, V)

    wd = w[:N_DENSE, 0]
    Vd = V[:, :N_DENSE].T.astype(np.float32)
    u = (Vd * Vd).sum(axis=1)
    rhs = np.zeros((KM, NO), dtype=np.float32)
    rhs[:N_DENSE, 0] = wd
    rhs[:N_DENSE, 1:ROW] = Vd
    rhs[N_DENSE : 2 * N_DENSE, ROW] = u
    rhs[2 * N_DENSE, 0] = w0[0]

    blk = (sparse // RPB).astype(np.int16)  # block-in-field
    cls = (sparse % RPB).astype(np.int64)  # class

    in_maps = []
    for core in range(N_CORES):
        dslice = dense[core * BL : (core + 1) * BL]
        dmat = np.empty((KM, BL + NO), dtype=np.float32)
        dmat[:N_DENSE, :BL] = dslice.T
        dmat[N_DENSE : 2 * N_DENSE, :BL] = (dslice * dslice).T
        dmat[2 * N_DENSE, :BL] = 1.0
        dmat[:, BL:] = rhs

        bslice = blk[core * BL : (core + 1) * BL]  # [512, 26]
        cslice = cls[core * BL : (core + 1) * BL]

        # per field: wrapped idx [128, 32]: item b -> [b%16, b//16], x8 groups
        idx_arr = np.empty((128, N_FIELDS * (BL // 16)), dtype=np.int16)
        for f in range(N_FIELDS):
            a = bslice[:, f].reshape(BL // 16, 16).T
            idx_arr[:, f * (BL // 16) : (f + 1) * (BL // 16)] = np.tile(a, (8, 1))

        # masks f32: per (h, c) a [T, FHS[h]] block; h0 blocks then h1 blocks
        cs = cslice.reshape(T, 128, N_FIELDS)  # [t, p, f]
        msk_arr = np.zeros((128, C * T * N_FIELDS), dtype=np.float32)
        for h in range(2):
            for c in range(C):
                sel = cs[:, :, F0S[h] : F0S[h] + FHS[h]] == c  # [t, p, fr]
                blk_ = sel.transpose(1, 0, 2).reshape(128, T * FHS[h])
                o = MOFF[h] + c * T * FHS[h]
                msk_arr[:, o : o + T * FHS[h]] = blk_

        in_maps.append({"table": table, "idx": idx_arr, "dmat": dmat, "msk": msk_arr})
    return in_maps


def assemble_output(results):
    out = np.empty((BATCH, 1), dtype=np.float32)
    for core in range(N_CORES):
        o = results[core]["out"]
        out[core * BL : (core + 1) * BL, 0] = o.T.reshape(BL)
    return out


_NC_CACHE = None


def kernel(**inputs) -> np.ndarray:
    global _NC_CACHE
    from concourse.bass_utils import run_bass_kernel_spmd

    if _NC_CACHE is None:
        _NC_CACHE = build_nc()
    nc = _NC_CACHE
    in_maps = prepare_inputs(**inputs)
    last_err = None
    for _ in range(3):
        try:
            res = run_bass_kernel_spmd(nc, in_maps, list(range(N_CORES)))
            return assemble_output(res.results)
        except Exception as e:  # noqa: BLE001
            last_err = e
    raise last_err



# revision 6
# speedup vs baseline: 1.2863x; 1.2863x over previous
"""FM layer (first + second order) on 8 TRN2 NeuronCores — fp16 dma_gather v2.

Batch-parallel (512 rows/core). Table rows are fp16 padded to 32 values
(64B): [w, V^T (16), ||V||^2, pad]; a 256B block holds 4 rows. One
dma_gather per field fetches each batch row's block via int16
block-in-field indices; gathers round-robin SWDGE queues 0-3 so the
~4.5us/gather Q7 descriptor generation overlaps 4-wide across core pairs
(0,1)/(2,3)/(4,5)/(6,7) — desc-gen latency, not DMA drain, paces the
pipeline. Row-within-block (class c = idx%4) is resolved by one DVE
tensor_tensor per quarter (all-fp16, j-replicated masks keep every AP
packed for 2x mode), a pair-sum tree, and an f32 tensor_reduce over fields.
The ||V||^2 table column makes Ssq a by-product of the same reduce,
eliminating v1's Act square+accum phase. Dense part and the final combine
reuse the packed-matmul trick (u folded into the normsq column).
"""

import os
import sys

sys.path.insert(0, "/opt/trn_rl_repo")

import numpy as np

import concourse.bass as bass
import concourse.bacc as bacc
import concourse.mybir as mybir
import concourse.tile as tile
from concourse import library_config
from concourse.ap import AP

N_DENSE = 13
N_FIELDS = 26
PER_FIELD = 100000
NROWS = N_FIELDS * PER_FIELD  # device table: sparse rows only
K = 16
BATCH = 4096
N_CORES = 8
BL = BATCH // N_CORES  # 512
P = 128
T = BL // P  # 4
ROW = 18  # w, V (16), ||V||^2
PAD = 32  # fp16 values per row (64B)
RPB = 4  # rows per 256B block
NBLK = PER_FIELD // RPB  # 25000
EW = RPB * PAD  # 128 fp16 words per block (256B)
C = RPB
FB = T * EW  # 512 fp16 words per field in HQ

# quarters (compute granularity; completes in gather issue order)
FQS = [8, 8, 8, 2]
FQ0 = [0, 8, 16, 24]
# mask word offsets per quarter: sum of C*T*FQ*ROW of previous quarters
QM0 = [0]
for _r in range(3):
    QM0.append(QM0[-1] + C * T * FQS[_r] * ROW)
MSKW = QM0[-1] + C * T * FQS[3] * ROW  # 7488

KM = 2 * N_DENSE + 1  # 27
NO = ROW  # matmul rhs columns (u folded into col 17)

F32 = mybir.dt.float32
F16 = mybir.dt.float16
I16 = mybir.dt.int16


def split_multiwaits(nc: bass.Bass, max_waits: int = 1) -> int:
    """This container's walrus encodes at most one sync-wait per instruction
    (setupSyncWait raises 'Too many sync wait commands' otherwise). Hoist
    extra waits into standalone EventSemaphore ops on the same engine.
    Each hoisted op incs a dedicated dummy sem nothing waits on (CoreSim
    requires EventSemaphore instructions to carry an update)."""
    import bass_rust

    used = set()
    for func in nc.m.functions:
        for bb in func.blocks:
            for ins in bb.instructions:
                si = getattr(ins, "sync_info", None)
                if si:
                    for x in list(si.on_wait or []) + list(si.on_update or []):
                        used.add(x.id)
    dummy = None
    for num in range(max(used, default=0) + 1, 256):
        try:
            dummy = nc.alloc_semaphore("splitw_dummy", num=num)
            break
        except AssertionError:
            continue
    assert dummy is not None, "no free semaphore for splitw_dummy"
    n = 0
    for func in nc.m.functions:
        for bb in func.blocks:
            out = []
            for ins in bb.instructions:
                si = getattr(ins, "sync_info", None)
                if (
                    si is not None
                    and si.on_wait is not None
                    and len(si.on_wait) > max_waits
                ):
                    for w in list(si.on_wait[:-max_waits]):
                        n += 1
                        ev = mybir.InstEventSemaphore(
                            name=f"splitw_{n}", engine=ins.engine
                        )
                        ev.sync_info = mybir.SyncInfo(on_wait=[w], on_update=[])
                        bass_rust.then_inc(ev, dummy, 1, True)
                        out.append(ev)
                    ins.sync_info = mybir.SyncInfo(
                        on_wait=list(si.on_wait[-max_waits:]),
                        on_update=list(si.on_update or []),
                    )
                out.append(ins)
            bb.instructions = out
    return n


def build_nc() -> bass.Bass:
    nc = bacc.Bacc("TRN2", num_swdge_queues=4, dynamic_dma_scratch_size=32768)

    table = nc.dram_tensor("table", [NROWS, PAD], F16, kind="ExternalInput")
    idx = nc.dram_tensor("idx", [128, N_FIELDS * (BL // 16)], I16, kind="ExternalInput")
    dmat = nc.dram_tensor("dmat", [KM, BL + NO], F32, kind="ExternalInput")
    msk = nc.dram_tensor("msk", [128, MSKW], F16, kind="ExternalInput")
    out = nc.dram_tensor("out", [P, T], F32, kind="ExternalOutput")

    # load the gather ucode first so the IRAM swap overlaps kernel startup
    nc.gpsimd.load_library(library_config.mlp)

    with tile.TileContext(nc) as tc:
        with (
            tc.tile_pool(name="const", bufs=1) as cp,
            tc.tile_pool(name="sbuf", bufs=1) as sp,
            tc.tile_pool(name="psum", bufs=1, space="PSUM") as pp,
        ):
            idx_t = cp.tile([128, N_FIELDS * (BL // 16)], I16)
            nc.sync.dma_start(idx_t[:], idx[:])
            dmat_t = cp.tile([KM, BL + NO], F32)
            nc.sync.dma_start(dmat_t[:], dmat[:])
            msk_t = cp.tile([128, MSKW], F16)
            nc.sync.dma_start(msk_t[:], msk[:])
            out_t = cp.tile([P, T], F32)

            mm_all = pp.tile([P, T * NO], F32)
            for t in range(T):
                nc.tensor.matmul(
                    mm_all[:, t * NO : (t + 1) * NO],
                    dmat_t[:, t * P : (t + 1) * P],
                    dmat_t[:, BL : BL + NO],
                    start=True,
                    stop=True,
                )

            HQ = [
                sp.tile([128, FQS[r] * FB], F16, tag=f"HQ{r}", name=f"HQ{r}")
                for r in range(4)
            ]
            # round-robin all 4 SWDGE queues: desc-gen (the ~4.5us/gather
            # serial cost per Q7 core pair) overlaps 4-wide across queue
            # pairs (0,1)/(2,3)/(4,5)/(6,7)
            for f in range(N_FIELDS):
                r = f // 8
                fr = f - FQ0[r]
                q = f % 4
                nc.gpsimd.dma_gather(
                    out_ap=HQ[r][:, fr * FB : (fr + 1) * FB].rearrange(
                        "p (s e) -> p s e", e=EW
                    ),
                    in_ap=AP(table, f * PER_FIELD * PAD, [[EW, NBLK], [1, EW]]),
                    idxs_ap=idx_t[:, f * (BL // 16) : (f + 1) * (BL // 16)],
                    num_idxs=BL,
                    num_idxs_reg=BL,
                    elem_size=EW,
                    elem_step=EW,
                    queue_num=q,
                )

            # per quarter: masked class-select (1 TT), pair-sum tree (2 TT),
            # f32 reduce over fields -> Sq [P, T*ROW]
            Sq = []
            for r in range(4):
                FQ = FQS[r]
                TF = T * FQ
                tmp = sp.tile([P, C * TF * ROW], F16, tag=f"tmp{r}", name=f"tmp{r}")
                # in0: HQ words (c, k=(f,t), j): k stride EW, c stride PAD
                in0 = AP(
                    HQ[r].tensor,
                    HQ[r][:].offset,
                    [HQ[r][:].ap[0], [PAD, C], [EW, TF], [1, ROW]],
                )
                in1 = AP(
                    msk_t.tensor,
                    msk_t[:].offset + QM0[r],
                    [msk_t[:].ap[0], [TF * ROW, C], [ROW, TF], [1, ROW]],
                )
                nc.vector.tensor_tensor(
                    out=tmp[:].rearrange("p (c x) -> p c x", c=C),
                    in0=in0,
                    in1=in1,
                    op=mybir.AluOpType.mult,
                )
                A = sp.tile([P, 2 * TF * ROW], F16, tag=f"A{r}", name=f"A{r}")
                nc.vector.tensor_tensor(
                    out=A[:],
                    in0=tmp[:, 0 : 2 * TF * ROW],
                    in1=tmp[:, 2 * TF * ROW : 4 * TF * ROW],
                    op=mybir.AluOpType.add,
                )
                B = sp.tile([P, TF * ROW], F16, tag=f"B{r}", name=f"B{r}")
                nc.vector.tensor_tensor(
                    out=B[:],
                    in0=A[:, 0 : TF * ROW],
                    in1=A[:, TF * ROW : 2 * TF * ROW],
                    op=mybir.AluOpType.add,
                )
                S = sp.tile([P, T * ROW], F32, tag=f"Sq{r}", name=f"Sq{r}")
                # B word = (f*T + t)*ROW + j ; reduce over f keeping (t, j)
                nc.vector.tensor_reduce(
                    out=S[:].rearrange("p (t j) -> p t j", t=T),
                    in_=AP(
                        B.tensor,
                        B[:].offset,
                        [B[:].ap[0], [ROW, T], [1, ROW], [T * ROW, FQ]],
                    ),
                    axis=mybir.AxisListType.X,
                    op=mybir.AluOpType.add,
                )
                Sq.append(S)

            Se01 = sp.tile([P, T * ROW], F32, tag="Se01", bufs=1)
            nc.vector.tensor_tensor(
                out=Se01[:], in0=Sq[0][:], in1=Sq[1][:], op=mybir.AluOpType.add
            )
            Se23 = sp.tile([P, T * ROW], F32, tag="Se23", bufs=1)
            nc.vector.tensor_tensor(
                out=Se23[:], in0=Sq[2][:], in1=Sq[3][:], op=mybir.AluOpType.add
            )
            Se = sp.tile([P, T * ROW], F32, tag="Se", bufs=1)
            nc.vector.tensor_tensor(
                out=Se[:], in0=Se01[:], in1=Se23[:], op=mybir.AluOpType.add
            )

            # ts = Se + mm ; per t: se2 = sum_k ts[1:17]^2 (Act square+accum)
            # out = ts[0] + 0.5*(se2 - ts[17])
            ts = sp.tile([P, T * ROW], F32, tag="ts", bufs=1)
            nc.vector.tensor_tensor(
                out=ts[:], in0=Se[:], in1=mm_all[:], op=mybir.AluOpType.add
            )
            se2 = sp.tile([P, T], F32, tag="se2", bufs=1)
            for t in range(T):
                sq2 = sp.tile([P, K], F32, tag="sq2", name=f"sq2{t}")
                nc.scalar.activation(
                    out=sq2[:],
                    in_=ts[:, t * ROW + 1 : t * ROW + 1 + K],
                    func=mybir.ActivationFunctionType.Square,
                    accum_out=se2[:, t : t + 1],
                )
            d1 = sp.tile([P, T], F32, tag="d1", bufs=1)
            nc.vector.tensor_tensor(
                out=d1[:],
                in0=se2[:],
                in1=AP(ts.tensor, ts[:].offset + (ROW - 1), [ts[:].ap[0], [ROW, T]]),
                op=mybir.AluOpType.subtract,
            )
            h1 = sp.tile([P, T], F32, tag="h1", bufs=1)
            nc.vector.tensor_scalar_mul(out=h1[:], in0=d1[:], scalar1=0.5)
            nc.vector.tensor_tensor(
                out=out_t[:],
                in0=h1[:],
                in1=AP(ts.tensor, ts[:].offset, [ts[:].ap[0], [ROW, T]]),
                op=mybir.AluOpType.add,
            )
            nc.sync.dma_start(out[:], out_t[:])

    split_multiwaits(nc)
    nc.compile()
    return nc


def prepare_table(w, V):
    """fp16 device table [NROWS, 32]: row j = [w, V, ||V||^2, pad].

    Rebuilt on every call: caching on first-call contents would silently
    return stale rows if the kernel is invoked again with different w/V."""
    tb = np.zeros((NROWS, PAD), dtype=np.float16)
    Vs = V[:, N_DENSE:]
    tb[:, 0] = w[N_DENSE:, 0]
    tb[:, 1 : 1 + K] = Vs.T
    tb[:, 1 + K] = (Vs.astype(np.float32) ** 2).sum(axis=0)
    return tb


def prepare_inputs(dense_inputs, sparse_inputs, w0, w, V):
    dense = np.asarray(dense_inputs, dtype=np.float32)
    sparse = np.asarray(sparse_inputs, dtype=np.int64)  # [B, 26] in [0, 1e5)
    w0 = np.asarray(w0, dtype=np.float32).reshape(-1)
    w = np.asarray(w, dtype=np.float32).reshape(-1, 1)
    V = np.asarray(V, dtype=np.float32)

    table = prepare_table(w, V)

    wd = w[:N_DENSE, 0]
    Vd = V[:, :N_DENSE].T.astype(np.float32)
    u = (Vd * Vd).sum(axis=1)
    rhs = np.zeros((KM, NO), dtype=np.float32)
    rhs[:N_DENSE, 0] = wd
    rhs[:N_DENSE, 1 : 1 + K] = Vd
    rhs[N_DENSE : 2 * N_DENSE, ROW - 1] = u
    rhs[2 * N_DENSE, 0] = w0[0]

    blk = (sparse // RPB).astype(np.int16)  # block-in-field
    cls = (sparse % RPB).astype(np.int64)  # class (row within block)

    in_maps = []
    for core in range(N_CORES):
        dslice = dense[core * BL : (core + 1) * BL]
        dmat = np.empty((KM, BL + NO), dtype=np.float32)
        dmat[:N_DENSE, :BL] = dslice.T
        dmat[N_DENSE : 2 * N_DENSE, :BL] = (dslice * dslice).T
        dmat[2 * N_DENSE, :BL] = 1.0
        dmat[:, BL:] = rhs

        bslice = blk[core * BL : (core + 1) * BL]  # [512, 26]
        cslice = cls[core * BL : (core + 1) * BL]

        # per field: wrapped idx [128, 32]: item b -> [b%16, b//16], x8 groups
        idx_arr = np.empty((128, N_FIELDS * (BL // 16)), dtype=np.int16)
        for f in range(N_FIELDS):
            a = bslice[:, f].reshape(BL // 16, 16).T
            idx_arr[:, f * (BL // 16) : (f + 1) * (BL // 16)] = np.tile(a, (8, 1))

        # masks fp16, j-replicated: per (quarter, class) a [FQ*T, ROW] block
        # word = QM0[r] + c*TF*ROW + (f_local*T + t)*ROW + j
        cs = cslice.reshape(T, 128, N_FIELDS)  # [t, p, f]
        msk_arr = np.zeros((128, MSKW), dtype=np.float16)
        for r in range(4):
            f0, FQ = FQ0[r], FQS[r]
            TF = T * FQ
            for c in range(C):
                sel = cs[:, :, f0 : f0 + FQ] == c  # [t, p, fl]
                blk_ = sel.transpose(1, 2, 0).reshape(128, TF)  # [p, fl*T + t]
                o = QM0[r] + c * TF * ROW
                msk_arr[:, o : o + TF * ROW] = np.repeat(
                    blk_.astype(np.float16), ROW, axis=1
                )

        in_maps.append({"table": table, "idx": idx_arr, "dmat": dmat, "msk": msk_arr})
    return in_maps


def assemble_output(results):
    out = np.empty((BATCH, 1), dtype=np.float32)
    for core in range(N_CORES):
        o = results[core]["out"]
        out[core * BL : (core + 1) * BL, 0] = o.T.reshape(BL)
    return out


_NC_CACHE = None


def kernel(**inputs) -> np.ndarray:
    global _NC_CACHE
    from concourse.bass_utils import run_bass_kernel_spmd

    if _NC_CACHE is None:
        _NC_CACHE = build_nc()
    nc = _NC_CACHE
    in_maps = prepare_inputs(**inputs)
    last_err = None
    for _ in range(3):
        try:
            res = run_bass_kernel_spmd(nc, in_maps, list(range(N_CORES)))
            return assemble_output(res.results)
        except Exception as e:  # noqa: BLE001
            last_err = e
    raise last_err
